# revision 5
# baseline (speedup 1.0000x reference)
"""Directed multi-head attention on 8 trn2 NeuronCores.

Sharding: 2-way batch x 4-way head tensor parallel. Core c handles batch
c//4 and heads [4*(c%4), 4*(c%4)+4) (256 of 1024 channels). Each core
computes its heads' q/k/v projections, causal attention, and a partial
out-projection; the host sums the 4 partials per batch (the all-reduce)
and adds b_o.

Layouts are chosen so every matmul contracts along the partition dim:
the host passes x^T, wq^T, wk^T, wv^T and w_o[:,S]^T per core. Scores are
computed transposed (keys on partitions, queries on free) so softmax's
denominator comes from a ones-column appended to V in the PV matmul, and
exp runs on [128, 1024] PSUM spans. All matmul operands are float32r
(tf32-like) which streams at full PE rate for free dims >= 256.
"""

import os

import numpy as np

E = 1024
H = 16
HD = 64
B = 2
NCORES = 8
GROUPS = NCORES // B  # head-parallel degree per batch
HPC = H // GROUPS  # heads per core
C = HPC * HD  # channels per core
P = 128
TI = 512  # i-tile (query) width, max fp32 matmul free dim

T = int(os.environ.get("ATTN_T", "2048"))
NEG = -1e9
SCALE = 1.0 / 8.0  # 1/sqrt(HD)

# fallback switches (flipped only if the toolchain rejects a path)
DVE_F32R = True  # DVE ops may write float32r directly
EXP_F32R = True  # ACT exp may write float32r directly

_cache = {}


def _build():
    import concourse.bacc as bacc
    import concourse.mybir as mybir
    from concourse.tile import TileContext

    F32 = mybir.dt.float32
    F32R = mybir.dt.float32r
    EXP = mybir.ActivationFunctionType.Exp
    COPY = mybir.ActivationFunctionType.Copy

    NT = T // TI  # i-tiles
    NJ = T // P  # j-tiles
    NE = E // P  # embed k-tiles
    NM = C // P  # channel partition tiles (2)

    nc = bacc.Bacc(debug=False)
    xT_d = nc.dram_tensor("xT", [E, T], F32R, kind="ExternalInput")
    wqT_d = nc.dram_tensor("wqT", [E, C], F32R, kind="ExternalInput")
    wkT_d = nc.dram_tensor("wkT", [E, C], F32R, kind="ExternalInput")
    wvT_d = nc.dram_tensor("wvT", [E, C], F32R, kind="ExternalInput")
    woT_d = nc.dram_tensor("woT", [C, E], F32R, kind="ExternalInput")
    cvT_d = nc.dram_tensor("cvT", [C, T], F32, kind="ExternalInput")
    mk_d = nc.dram_tensor("mk", [4 * P, TI], F32, kind="ExternalInput")
    ones_d = nc.dram_tensor("ones", [P, HPC], F32R, kind="ExternalInput")
    poT_d = nc.dram_tensor("poT", [E, T], F32, kind="ExternalOutput")

    with TileContext(nc) as tc:
        with (
            tc.tile_pool(name="persist", bufs=1) as pp,
            tc.tile_pool(name="scratch", bufs=2) as scr,
        ):
            # persistent tiles (live across both phases)
            qT = [pp.tile([P, T], F32R, tag=f"qT{m}", name=f"qT{m}") for m in range(NM)]
            kT = [pp.tile([P, T], F32R, tag=f"kT{m}", name=f"kT{m}") for m in range(NM)]
            vg = [pp.tile([P, HPC, HD + 1], F32R, tag=f"vg{j}", name=f"vg{j}") for j in range(NJ)]
            for j in range(NJ):
                nc.sync.dma_start(out=vg[j][:, :, HD : HD + 1], in_=ones_d[:, :].unsqueeze(2))

            # ---------------- phase 1: projections ----------------
            with (
                tc.tile_pool(name="proj_in", bufs=1) as wp,
                tc.tile_pool(name="cv_in", bufs=3) as cvp,
                tc.tile_pool(name="proj_ps", bufs=2, space="PSUM") as pps,
            ):
                xT = [wp.tile([P, T], F32R, tag=f"xT{e}", name=f"xT{e}") for e in range(NE)]
                for e in range(NE):
                    nc.sync.dma_start(out=xT[e], in_=xT_d[P * e : P * e + P, :])
                wq = [wp.tile([P, C], F32R, tag=f"wq{e}", name=f"wq{e}") for e in range(NE)]
                wk = [wp.tile([P, C], F32R, tag=f"wk{e}", name=f"wk{e}") for e in range(NE)]
                wv = [wp.tile([P, C], F32R, tag=f"wv{e}", name=f"wv{e}") for e in range(NE)]
                for e in range(NE):
                    nc.sync.dma_start(out=wq[e], in_=wqT_d[P * e : P * e + P, :])
                    nc.sync.dma_start(out=wk[e], in_=wkT_d[P * e : P * e + P, :])
                    nc.sync.dma_start(out=wv[e], in_=wvT_d[P * e : P * e + P, :])

                def evict_f32r_add(dst, src_ps, add=None):
                    # dst: f32r sbuf AP; src_ps: f32 psum AP
                    if DVE_F32R:
                        if add is not None:
                            nc.vector.tensor_add(out=dst, in0=src_ps, in1=add)
                        else:
                            nc.vector.tensor_copy(out=dst, in_=src_ps)
                    else:
                        tmp = scr.tile([P, TI], F32, tag="evtmp", name="evtmp")
                        tslc = tmp[: dst.shape[0], : src_ps.free_size()]
                        if add is not None:
                            nc.vector.tensor_add(out=tslc, in0=src_ps, in1=add)
                        else:
                            nc.vector.tensor_copy(out=tslc, in_=src_ps)
                        nc.scalar.activation(out=dst, in_=tslc, func=COPY)

                # q^T and k^T: [C, T] = wT.T @ xT
                for m in range(NM):
                    for n in range(NT):
                        ns = slice(TI * n, TI * n + TI)
                        cvt = cvp.tile([P, TI], F32, tag="cv", name="cv")
                        nc.sync.dma_start(out=cvt, in_=cvT_d[P * m : P * m + P, ns])
                        psq = pps.tile([P, TI], F32, tag="pjq", name="pjq")
                        psk = pps.tile([P, TI], F32, tag="pjk", name="pjk")
                        for e in range(NE):
                            nc.tensor.matmul(
                                psq, wq[e][:, P * m : P * m + P], xT[e][:, ns],
                                start=(e == 0), stop=(e == NE - 1),
                            )
                        for e in range(NE):
                            nc.tensor.matmul(
                                psk, wk[e][:, P * m : P * m + P], xT[e][:, ns],
                                start=(e == 0), stop=(e == NE - 1),
                            )
                        evict_f32r_add(qT[m][:, ns], psq, add=cvt)
                        evict_f32r_add(kT[m][:, ns], psk)
                # v: [T, C] = xT.T @ wvT, scattered into vg (65-stride head groups)
                for j in range(NJ):
                    psv = pps.tile([P, C], F32, tag="pjv", name="pjv")
                    for e in range(NE):
                        nc.tensor.matmul(
                            psv, xT[e][:, P * j : P * j + P], wv[e],
                            start=(e == 0), stop=(e == NE - 1),
                        )
                    evict_f32r_add(
                        vg[j][:, :, 0:HD],
                        psv.rearrange("p (h c) -> p h c", c=HD),
                    )

            # ---------------- phase 2: attention + out-proj ----------------
            with (
                tc.tile_pool(name="attn_sb", bufs=1) as ap2,
                tc.tile_pool(name="sc_ps", bufs=2, space="PSUM") as scp,
                tc.tile_pool(name="out_ps", bufs=2, space="PSUM") as outp,
                tc.tile_pool(name="op_ps", bufs=2, space="PSUM") as opp,
                tc.tile_pool(name="zp", bufs=4) as zp,
                tc.tile_pool(name="np_", bufs=4) as np_,
                tc.tile_pool(name="po", bufs=4) as pop,
            ):
                hoT = [ap2.tile([P, T], F32R, tag=f"hoT{m}", name=f"hoT{m}") for m in range(NM)]
                mkt = [ap2.tile([P, TI], F32, tag=f"mk{s}", name=f"mk{s}") for s in range(4)]
                woT = [ap2.tile([P, E], F32R, tag=f"woT{m}", name=f"woT{m}") for m in range(NM)]
                for s in range(4):
                    nc.sync.dma_start(out=mkt[s], in_=mk_d[P * s : P * s + P, :])
                for m in range(NM):
                    nc.sync.dma_start(out=woT[m], in_=woT_d[P * m : P * m + P, :])
                for it in range(NT):
                    isl = slice(TI * it, TI * it + TI)
                    njt = 4 * (it + 1)
                    for hp in range(NM):
                        m = hp
                        outps = [outp.tile([P, TI], F32, tag="out", name="out") for _ in range(2)]
                        first_pv = [True, True]
                        for g in range(njt // 2):
                            jts = (2 * g, 2 * g + 1)
                            scs = [scp.tile([P, 2 * TI], F32, tag="sc", name="sc") for _ in range(2)]
                            # row-packed score matmuls: heads at base partitions 0/64
                            for jj, jt in enumerate(jts):
                                for hx in range(2):
                                    bp = 64 * hx
                                    nc.tensor.matmul(
                                        scs[hx][:, TI * jj : TI * jj + TI],
                                        kT[m][bp : bp + 64, P * jt : P * jt + P],
                                        qT[m][bp : bp + 64, isl],
                                        start=True, stop=True,
                                    )
                            # causal mask (additive -1e9) on partial j-tiles
                            for jj, jt in enumerate(jts):
                                s = jt - 4 * it
                                if 0 <= s <= 3:
                                    for hx in range(2):
                                        slc = scs[hx][:, TI * jj : TI * jj + TI]
                                        nc.vector.tensor_add(out=slc, in0=slc, in1=mkt[s])
                            # exp over the 2-jt span
                            zs = []
                            for hx in range(2):
                                if EXP_F32R:
                                    z = zp.tile([P, 2 * TI], F32R, tag="z", name="z")
                                    nc.scalar.activation(out=z, in_=scs[hx], func=EXP, scale=SCALE)
                                else:
                                    zf = zp.tile([P, 2 * TI], F32, tag="zf", name="zf")
                                    nc.scalar.activation(out=zf, in_=scs[hx], func=EXP, scale=SCALE)
                                    z = zp.tile([P, 2 * TI], F32R, tag="z", name="z")
                                    nc.vector.tensor_copy(out=z, in_=zf)
                                zs.append(z)
                            # PV accumulate (+ ones column -> denominator row 64)
                            for jj, jt in enumerate(jts):
                                for hx in range(2):
                                    h = 2 * hp + hx
                                    nc.tensor.matmul(
                                        outps[hx][0 : HD + 1, :],
                                        vg[jt].rearrange("p h c -> p (h c)")[:, 65 * h : 65 * h + 65],
                                        zs[hx][:, TI * jj : TI * jj + TI],
                                        start=first_pv[hx], stop=(g == njt // 2 - 1 and jj == 1),
                                    )
                                    first_pv[hx] = False
                        # normalize: rows/denominator-row -> hoT
                        for hx in range(2):
                            rec = np_.tile([1, TI], F32, tag="rec", name="rec")
                            nc.vector.reciprocal(out=rec, in_=outps[hx][HD : HD + 1, :])
                            recb = np_.tile([HD, TI], F32, tag="recb", name="recb")
                            nc.gpsimd.partition_broadcast(recb, rec)
                            dst = hoT[m][HD * hx : HD * hx + HD, isl]
                            if DVE_F32R:
                                nc.vector.tensor_mul(out=dst, in0=outps[hx][0:HD, :], in1=recb)
                            else:
                                tmp = np_.tile([HD, TI], F32, tag="ntmp", name="ntmp")
                                nc.vector.tensor_mul(out=tmp, in0=outps[hx][0:HD, :], in1=recb)
                                nc.scalar.activation(out=dst, in_=tmp, func=COPY)
                    # out-projection for this i-tile
                    for mo in range(NE):
                        ps = opp.tile([P, TI], F32, tag="op", name="op")
                        for ki in range(NM):
                            nc.tensor.matmul(
                                ps, woT[ki][:, P * mo : P * mo + P], hoT[ki][:, isl],
                                start=(ki == 0), stop=(ki == NM - 1),
                            )
                        ev = pop.tile([P, TI], F32, tag="po", name="po")
                        nc.vector.tensor_copy(out=ev, in_=ps)
                        nc.sync.dma_start(out=poT_d[P * mo : P * mo + P, isl], in_=ev)

    nc.finalize()
    return nc


def _get_nc():
    if "nc" not in _cache:
        _cache["nc"] = _build()
    return _cache["nc"]


def _masks():
    # mk[s][p, f] = 0 if f >= p + 128*s else -1e9   (allowed iff j <= i)
    f = np.arange(TI)[None, :]
    p = np.arange(P)[:, None]
    out = np.zeros((4 * P, TI), dtype=np.float32)
    for s in range(4):
        out[P * s : P * s + P] = np.where(f >= p + P * s, 0.0, NEG)
    return out


def kernel(x, mask, context_vector, w_q, w_k, w_v, w_o, b_o, context_bias):
    from concourse.bass_utils import run_bass_kernel_spmd

    x = np.asarray(x, dtype=np.float32)
    context_vector = np.asarray(context_vector, dtype=np.float32)
    w_q = np.asarray(w_q, dtype=np.float32)
    w_k = np.asarray(w_k, dtype=np.float32)
    w_v = np.asarray(w_v, dtype=np.float32)
    w_o = np.asarray(w_o, dtype=np.float32)
    b_o = np.asarray(b_o, dtype=np.float32)
    context_bias = np.asarray(context_bias, dtype=np.float32)

    nc = _get_nc()
    mk = _masks()
    ones = np.ones((P, HPC), dtype=np.float32)
    bcv = context_bias[None, None, :] * context_vector  # [B, T, E]

    in_maps = []
    for c in range(NCORES):
        b, g = divmod(c, GROUPS)
        S = slice(C * g, C * g + C)
        in_maps.append({
            "xT": np.ascontiguousarray(x[b, :T].T),
            "wqT": np.ascontiguousarray(w_q[S, :].T),
            "wkT": np.ascontiguousarray(w_k[S, :].T),
            "wvT": np.ascontiguousarray(w_v[S, :].T),
            "woT": np.ascontiguousarray(w_o[:, S].T),
            "cvT": np.ascontiguousarray(bcv[b, :T, S].T),
            "mk": mk,
            "ones": ones,
        })

    res = run_bass_kernel_spmd(
        nc, in_maps, list(range(NCORES)),
        trace=bool(int(os.environ.get("ATTN_TRACE", "0"))),
    )
    _cache["last_results"] = res

    out = np.zeros((B, T, E), dtype=np.float32)
    for c in range(NCORES):
        b = c // GROUPS
        out[b] += res.results[c]["poT"].T
    out += b_o[None, None, :]
    return out


# revision 7
# speedup vs baseline: 1.1052x; 1.1052x over previous
"""Directed multi-head attention on 8 trn2 NeuronCores.

Sharding: 2-way batch x 4-way head tensor parallel. Core c handles batch
c//4 and heads [4*(c%4), 4*(c%4)+4) (256 of 1024 channels). Each core
computes its heads' q/k/v projections, causal attention, and a partial
out-projection; the host sums the 4 partials per batch (the all-reduce)
and adds b_o.

Layouts are chosen so every matmul contracts along the partition dim:
the host passes x^T, wq^T, wk^T, wv^T and w_o[:,S]^T per core. Scores are
computed transposed (keys on partitions, queries on free) so softmax's
denominator comes from a ones-column appended to V in the PV matmul, and
exp runs on [128, 1024] PSUM spans. All matmul operands are float32r
(tf32-like) which streams at full PE rate for free dims >= 256.

The q/k/v projections are software-pipelined INTO the attention i-tile
loop (projections for i-tile n+1 are emitted between attention groups of
i-tile n) so the PE always has independent work while the Scalar engine
runs exp -- otherwise the PE HAM clock-gate re-throttles to 1.2 GHz
during ACT-bound stretches and doubles every matmul.
"""

import os

import numpy as np

E = 1024
H = 16
HD = 64
B = 2
NCORES = 8
GROUPS = NCORES // B  # head-parallel degree per batch
HPC = H // GROUPS  # heads per core
C = HPC * HD  # channels per core
P = 128
TI = 512  # i-tile (query) width, max fp32 matmul free dim

T = int(os.environ.get("ATTN_T", "2048"))
NEG = -1e9
SCALE = 1.0 / 8.0  # 1/sqrt(HD)

_cache = {}


def _build():
    import concourse.bacc as bacc
    import concourse.mybir as mybir
    from concourse.tile import TileContext

    F32 = mybir.dt.float32
    F32R = mybir.dt.float32r
    EXP = mybir.ActivationFunctionType.Exp

    NT = T // TI  # i-tiles
    NJ = T // P  # j-tiles
    NE = E // P  # embed k-tiles
    NM = C // P  # channel partition tiles (2)

    nc = bacc.Bacc(debug=False)
    xT_d = nc.dram_tensor("xT", [E, T], F32R, kind="ExternalInput")
    wqT_d = nc.dram_tensor("wqT", [E, C], F32R, kind="ExternalInput")
    wkT_d = nc.dram_tensor("wkT", [E, C], F32R, kind="ExternalInput")
    wvT_d = nc.dram_tensor("wvT", [E, C], F32R, kind="ExternalInput")
    woT_d = nc.dram_tensor("woT", [C, E], F32R, kind="ExternalInput")
    cvT_d = nc.dram_tensor("cvT", [C, T], F32, kind="ExternalInput")
    mk_d = nc.dram_tensor("mk", [4 * P, TI], F32, kind="ExternalInput")
    ones_d = nc.dram_tensor("ones", [P, HPC], F32R, kind="ExternalInput")
    poT_d = nc.dram_tensor("poT", [E, T], F32, kind="ExternalOutput")

    with TileContext(nc) as tc:
        with (
            tc.tile_pool(name="persist", bufs=1) as pp,
            tc.tile_pool(name="xs", bufs=2) as xsp,
            tc.tile_pool(name="cv_in", bufs=3) as cvp,
            tc.tile_pool(name="zp", bufs=4) as zp,
            tc.tile_pool(name="np_", bufs=4) as np_,
            tc.tile_pool(name="po", bufs=4) as pop,
            tc.tile_pool(name="sc_ps", bufs=2, space="PSUM") as scp,
            tc.tile_pool(name="out_ps", bufs=2, space="PSUM") as outp,
            tc.tile_pool(name="aux_ps", bufs=2, space="PSUM") as auxp,
        ):
            # ---- persistent tiles
            qT = [pp.tile([P, T], F32R, tag=f"qT{m}", name=f"qT{m}") for m in range(NM)]
            kT = [pp.tile([P, T], F32R, tag=f"kT{m}", name=f"kT{m}") for m in range(NM)]
            vg = [pp.tile([P, HPC, HD + 1], F32R, tag=f"vg{j}", name=f"vg{j}") for j in range(NJ)]
            hoT = [pp.tile([P, T], F32R, tag=f"hoT{m}", name=f"hoT{m}") for m in range(NM)]
            mkt = [pp.tile([P, TI], F32, tag=f"mk{s}", name=f"mk{s}") for s in range(4)]
            woT = [pp.tile([P, E], F32R, tag=f"woT{m}", name=f"woT{m}") for m in range(NM)]
            wq = [pp.tile([P, C], F32R, tag=f"wq{e}", name=f"wq{e}") for e in range(NE)]
            wk = [pp.tile([P, C], F32R, tag=f"wk{e}", name=f"wk{e}") for e in range(NE)]
            wv = [pp.tile([P, C], F32R, tag=f"wv{e}", name=f"wv{e}") for e in range(NE)]

            for j in range(NJ):
                nc.sync.dma_start(out=vg[j][:, :, HD : HD + 1], in_=ones_d[:, :].unsqueeze(2))
            for s in range(4):
                nc.sync.dma_start(out=mkt[s], in_=mk_d[P * s : P * s + P, :])
            for m in range(NM):
                nc.sync.dma_start(out=woT[m], in_=woT_d[P * m : P * m + P, :])
            for e in range(NE):
                nc.sync.dma_start(out=wq[e], in_=wqT_d[P * e : P * e + P, :])
                nc.sync.dma_start(out=wk[e], in_=wkT_d[P * e : P * e + P, :])
                nc.sync.dma_start(out=wv[e], in_=wvT_d[P * e : P * e + P, :])

            # ---- projection emitters (generators: one yield = one PE-chunk)
            def proj_steps(n):
                """Emit q/k/v projections for token window n (cols TI*n..) in
                small chunks; yields between chunks so the caller can
                interleave them with attention groups."""
                ns = slice(TI * n, TI * n + TI)
                xs = [xsp.tile([P, TI], F32R, tag=f"xs{e}", name=f"xs{e}") for e in range(NE)]
                for e in range(NE):
                    nc.sync.dma_start(out=xs[e], in_=xT_d[P * e : P * e + P, ns])
                yield
                for m in range(NM):
                    cvt = cvp.tile([P, TI], F32, tag="cv", name="cv")
                    nc.sync.dma_start(out=cvt, in_=cvT_d[P * m : P * m + P, ns])
                    psq = auxp.tile([P, TI], F32, tag="aux", name="psq")
                    for e in range(NE):
                        nc.tensor.matmul(
                            psq, wq[e][:, P * m : P * m + P], xs[e],
                            start=(e == 0), stop=(e == NE - 1),
                        )
                    nc.vector.tensor_add(out=qT[m][:, ns], in0=psq, in1=cvt)
                    yield
                    psk = auxp.tile([P, TI], F32, tag="aux", name="psk")
                    for e in range(NE):
                        nc.tensor.matmul(
                            psk, wk[e][:, P * m : P * m + P], xs[e],
                            start=(e == 0), stop=(e == NE - 1),
                        )
                    nc.vector.tensor_copy(out=kT[m][:, ns], in_=psk)
                    yield
                for jj in range(TI // P):
                    j = (TI * n) // P + jj
                    psv = auxp.tile([P, C], F32, tag="aux", name="psv")
                    for e in range(NE):
                        nc.tensor.matmul(
                            psv, xs[e][:, P * jj : P * jj + P], wv[e],
                            start=(e == 0), stop=(e == NE - 1),
                        )
                    nc.vector.tensor_copy(
                        out=vg[j][:, :, 0:HD],
                        in_=psv.rearrange("p (h c) -> p h c", c=HD),
                    )
                    yield

            def drain(gen):
                if gen is not None:
                    for _ in gen:
                        pass

            def step(gen, k=1):
                if gen is None:
                    return
                for _ in range(k):
                    if next(gen, "done") == "done":
                        return

            # ---- prologue: projections for i-tile 0
            drain(proj_steps(0))

            # ---- main loop: attention(it) + pipelined projections(it+1)
            for it in range(NT):
                isl = slice(TI * it, TI * it + TI)
                njt = 4 * (it + 1)
                filler = proj_steps(it + 1) if it + 1 < NT else None
                for hp in range(NM):
                    m = hp
                    outps = [
                        outp.tile([P, TI], F32, tag="out", name=f"out{hx}")
                        for hx in range(2)
                    ]
                    for g in range(njt // 2):
                        jts = (2 * g, 2 * g + 1)
                        scs = [
                            scp.tile([P, 2 * TI], F32, tag="sc", name=f"sc{hx}")
                            for hx in range(2)
                        ]
                        # row-packed score matmuls: heads at base partitions 0/64
                        for jj, jt in enumerate(jts):
                            for hx in range(2):
                                bp = 64 * hx
                                nc.tensor.matmul(
                                    scs[hx][:, TI * jj : TI * jj + TI],
                                    kT[m][bp : bp + 64, P * jt : P * jt + P],
                                    qT[m][bp : bp + 64, isl],
                                    start=True, stop=True,
                                )
                        # causal mask (additive -1e9) on partial j-tiles
                        for jj, jt in enumerate(jts):
                            s = jt - 4 * it
                            if 0 <= s <= 3:
                                for hx in range(2):
                                    slc = scs[hx][:, TI * jj : TI * jj + TI]
                                    nc.vector.tensor_add(out=slc, in0=slc, in1=mkt[s])
                        # exp over the 2-jt span
                        zs = []
                        for hx in range(2):
                            z = zp.tile([P, 2 * TI], F32R, tag="z", name="z")
                            nc.scalar.activation(out=z, in_=scs[hx], func=EXP, scale=SCALE)
                            zs.append(z)
                        # PV accumulate (+ ones column -> denominator row HD)
                        for jj, jt in enumerate(jts):
                            for hx in range(2):
                                h = 2 * hp + hx
                                nc.tensor.matmul(
                                    outps[hx][0 : HD + 1, :],
                                    vg[jt].rearrange("p h c -> p (h c)")[:, 65 * h : 65 * h + 65],
                                    zs[hx][:, TI * jj : TI * jj + TI],
                                    start=(g == 0 and jj == 0),
                                    stop=(g == njt // 2 - 1 and jj == 1),
                                )
                        # independent projection work to keep the PE warm
                        step(filler)
                    # normalize rows by the denominator row -> hoT
                    for hx in range(2):
                        rec = np_.tile([1, TI], F32, tag="rec", name="rec")
                        nc.vector.reciprocal(out=rec, in_=outps[hx][HD : HD + 1, :])
                        recb = np_.tile([HD, TI], F32, tag="recb", name="recb")
                        nc.gpsimd.partition_broadcast(recb, rec)
                        nc.vector.tensor_mul(
                            out=hoT[m][HD * hx : HD * hx + HD, isl],
                            in0=outps[hx][0:HD, :], in1=recb,
                        )
                # out-projection for this i-tile
                for mo in range(NE):
                    ps = auxp.tile([P, TI], F32, tag="aux", name="op")
                    for ki in range(NM):
                        nc.tensor.matmul(
                            ps, woT[ki][:, P * mo : P * mo + P], hoT[ki][:, isl],
                            start=(ki == 0), stop=(ki == NM - 1),
                        )
                    ev = pop.tile([P, TI], F32, tag="po", name="po")
                    nc.vector.tensor_copy(out=ev, in_=ps)
                    nc.sync.dma_start(out=poT_d[P * mo : P * mo + P, isl], in_=ev)
                    if mo % 4 == 3:
                        step(filler)
                drain(filler)

    nc.finalize()
    return nc


def _get_nc():
    if "nc" not in _cache:
        _cache["nc"] = _build()
    return _cache["nc"]


def _masks():
    # mk[s][p, f] = 0 if f >= p + 128*s else -1e9   (allowed iff j <= i)
    f = np.arange(TI)[None, :]
    p = np.arange(P)[:, None]
    out = np.zeros((4 * P, TI), dtype=np.float32)
    for s in range(4):
        out[P * s : P * s + P] = np.where(f >= p + P * s, 0.0, NEG)
    return out


def kernel(x, mask, context_vector, w_q, w_k, w_v, w_o, b_o, context_bias):
    from concourse.bass_utils import run_bass_kernel_spmd

    x = np.asarray(x, dtype=np.float32)
    context_vector = np.asarray(context_vector, dtype=np.float32)
    w_q = np.asarray(w_q, dtype=np.float32)
    w_k = np.asarray(w_k, dtype=np.float32)
    w_v = np.asarray(w_v, dtype=np.float32)
    w_o = np.asarray(w_o, dtype=np.float32)
    b_o = np.asarray(b_o, dtype=np.float32)
    context_bias = np.asarray(context_bias, dtype=np.float32)

    nc = _get_nc()
    mk = _masks()
    ones = np.ones((P, HPC), dtype=np.float32)
    bcv = context_bias[None, None, :] * context_vector  # [B, T, E]

    in_maps = []
    for c in range(NCORES):
        b, g = divmod(c, GROUPS)
        S = slice(C * g, C * g + C)
        in_maps.append({
            "xT": np.ascontiguousarray(x[b, :T].T),
            "wqT": np.ascontiguousarray(w_q[S, :].T),
            "wkT": np.ascontiguousarray(w_k[S, :].T),
            "wvT": np.ascontiguousarray(w_v[S, :].T),
            "woT": np.ascontiguousarray(w_o[:, S].T),
            "cvT": np.ascontiguousarray(bcv[b, :T, S].T),
            "mk": mk,
            "ones": ones,
        })

    res = run_bass_kernel_spmd(
        nc, in_maps, list(range(NCORES)),
        trace=bool(int(os.environ.get("ATTN_TRACE", "0"))),
    )
    _cache["last_results"] = res

    out = np.zeros((B, T, E), dtype=np.float32)
    for c in range(NCORES):
        b = c // GROUPS
        out[b] += res.results[c]["poT"].T
    out += b_o[None, None, :]
    return out


# revision 14
# speedup vs baseline: 1.1837x; 1.0711x over previous
"""Directed multi-head attention on 8 trn2 NeuronCores.

Sharding: 2-way batch x 4-way head tensor parallel. Core c handles batch
c//4 and heads [4*(c%4), 4*(c%4)+4) (256 of 1024 channels). Each core
computes its heads' q/k/v projections, causal attention, and a partial
out-projection; the host sums the 4 partials per batch (the all-reduce)
and adds b_o.

Layouts are chosen so every matmul contracts along the partition dim:
the host passes x^T, wq^T, wk^T, wv^T and w_o[:,S]^T per core. Scores are
computed transposed (keys on partitions, queries on free) so softmax's
denominator comes from a ones-column appended to V in the PV matmul, and
exp runs on [128, 1024] PSUM spans. All matmul operands are float32r
(tf32-like) which streams at full PE rate for free dims >= 256.

The q/k/v projections are software-pipelined INTO the attention i-tile
loop (projections for i-tile n+1 are emitted between attention groups of
i-tile n) so the PE always has independent work while the Scalar engine
runs exp -- otherwise the PE HAM clock-gate re-throttles to 1.2 GHz
during ACT-bound stretches and doubles every matmul.
"""

import os

import numpy as np

E = 1024
H = 16
HD = 64
B = 2
NCORES = 8
GROUPS = NCORES // B  # head-parallel degree per batch
HPC = H // GROUPS  # heads per core
C = HPC * HD  # channels per core
P = 128
TI = 512  # i-tile (query) width, max fp32 matmul free dim

T = int(os.environ.get("ATTN_T", "2048"))
NEG = -1e9
SCALE = 1.0 / 8.0  # 1/sqrt(HD)

_cache = {}


def _build():
    import concourse.bacc as bacc
    import concourse.mybir as mybir
    from concourse.tile import TileContext

    F32 = mybir.dt.float32
    F32R = mybir.dt.float32r
    EXP = mybir.ActivationFunctionType.Exp
    COPY = mybir.ActivationFunctionType.Copy

    NT = T // TI  # i-tiles
    NJ = T // P  # j-tiles
    NE = E // P  # embed k-tiles
    NM = C // P  # channel partition tiles (2)

    nc = bacc.Bacc(debug=False)
    xT_d = nc.dram_tensor("xT", [E, T], F32R, kind="ExternalInput")
    wqT_d = nc.dram_tensor("wqT", [E, C], F32R, kind="ExternalInput")
    wkT_d = nc.dram_tensor("wkT", [E, C], F32R, kind="ExternalInput")
    wvT_d = nc.dram_tensor("wvT", [E, C], F32R, kind="ExternalInput")
    woT_d = nc.dram_tensor("woT", [C, E], F32R, kind="ExternalInput")
    cvT_d = nc.dram_tensor("cvT", [C, T], F32, kind="ExternalInput")
    mk_d = nc.dram_tensor("mk", [4 * P, TI], F32, kind="ExternalInput")
    ones_d = nc.dram_tensor("ones", [P, HPC], F32R, kind="ExternalInput")
    poT_d = nc.dram_tensor("poT", [E, T], F32, kind="ExternalOutput")

    with TileContext(nc) as tc:
        with (
            tc.tile_pool(name="persist", bufs=1) as pp,
            tc.tile_pool(name="xs", bufs=2) as xsp,
            tc.tile_pool(name="cv_in", bufs=3) as cvp,
            tc.tile_pool(name="zp", bufs=4) as zp,
            tc.tile_pool(name="np_", bufs=4) as np_,
            tc.tile_pool(name="po", bufs=4) as pop,
            tc.tile_pool(name="sc_ps", bufs=2, space="PSUM") as scp,
            tc.tile_pool(name="out_ps", bufs=2, space="PSUM") as outp,
            tc.tile_pool(name="aux_ps", bufs=2, space="PSUM") as auxp,
        ):
            # ---- persistent tiles
            qT = [pp.tile([P, T], F32R, tag=f"qT{m}", name=f"qT{m}") for m in range(NM)]
            kT = [pp.tile([P, T], F32R, tag=f"kT{m}", name=f"kT{m}") for m in range(NM)]
            vg = [pp.tile([P, HPC, HD + 1], F32R, tag=f"vg{j}", name=f"vg{j}") for j in range(NJ)]
            hoT = [pp.tile([P, T], F32R, tag=f"hoT{m}", name=f"hoT{m}") for m in range(NM)]
            mkt = [pp.tile([P, TI], F32, tag=f"mk{s}", name=f"mk{s}") for s in range(4)]
            woT = [pp.tile([P, E], F32R, tag=f"woT{m}", name=f"woT{m}") for m in range(NM)]
            wq = [pp.tile([P, C], F32R, tag=f"wq{e}", name=f"wq{e}") for e in range(NE)]
            wk = [pp.tile([P, C], F32R, tag=f"wk{e}", name=f"wk{e}") for e in range(NE)]
            wv = [pp.tile([P, C], F32R, tag=f"wv{e}", name=f"wv{e}") for e in range(NE)]

            # weights on the scalar HWDGE queue, streaming data on sync, tiny
            # ones-column writes on gpsimd SWDGE: the prologue was previously
            # ~40us of serialized DMA on one queue.
            for e in range(NE):
                nc.scalar.dma_start(out=wq[e], in_=wqT_d[P * e : P * e + P, :])
                nc.scalar.dma_start(out=wk[e], in_=wkT_d[P * e : P * e + P, :])
                nc.scalar.dma_start(out=wv[e], in_=wvT_d[P * e : P * e + P, :])
            for j in range(NJ):
                nc.gpsimd.dma_start(out=vg[j][:, :, HD : HD + 1], in_=ones_d[:, :].unsqueeze(2))
            for s in range(4):
                nc.scalar.dma_start(out=mkt[s], in_=mk_d[P * s : P * s + P, :])
            for m in range(NM):
                nc.scalar.dma_start(out=woT[m], in_=woT_d[P * m : P * m + P, :])
            # warm the ACT exp spline table before the first real exp
            warm = pp.tile([1, 16], F32, tag="warm", name="warm")
            nc.vector.memset(warm, 0.0)
            warm2 = pp.tile([1, 16], F32, tag="warm2", name="warm2")
            nc.scalar.activation(out=warm2, in_=warm, func=EXP)

            # ---- projection emitters (generators: one yield = one PE-chunk)
            def proj_steps(n):
                """Emit q/k/v projections for token window n (cols TI*n..) in
                small chunks; yields between chunks so the caller can
                interleave them with attention groups."""
                ns = slice(TI * n, TI * n + TI)
                xs = [xsp.tile([P, TI], F32R, tag=f"xs{e}", name=f"xs{e}") for e in range(NE)]
                for e in range(NE):
                    nc.sync.dma_start(out=xs[e], in_=xT_d[P * e : P * e + P, ns])
                yield
                for m in range(NM):
                    cvt = cvp.tile([P, TI], F32, tag="cv", name="cv")
                    nc.sync.dma_start(out=cvt, in_=cvT_d[P * m : P * m + P, ns])
                    psq = auxp.tile([P, TI], F32, tag="aux", name="psq")
                    for e in range(NE):
                        nc.tensor.matmul(
                            psq, wq[e][:, P * m : P * m + P], xs[e],
                            start=(e == 0), stop=(e == NE - 1),
                        )
                    nc.vector.tensor_add(out=qT[m][:, ns], in0=psq, in1=cvt)
                    yield
                    psk = auxp.tile([P, TI], F32, tag="aux", name="psk")
                    for e in range(NE):
                        nc.tensor.matmul(
                            psk, wk[e][:, P * m : P * m + P], xs[e],
                            start=(e == 0), stop=(e == NE - 1),
                        )
                    nc.vector.tensor_copy(out=kT[m][:, ns], in_=psk)
                    yield
                for jj in range(TI // P):
                    j = (TI * n) // P + jj
                    psv = auxp.tile([P, C], F32, tag="aux", name="psv")
                    for e in range(NE):
                        nc.tensor.matmul(
                            psv, xs[e][:, P * jj : P * jj + P], wv[e],
                            start=(e == 0), stop=(e == NE - 1),
                        )
                    nc.vector.tensor_copy(
                        out=vg[j][:, :, 0:HD],
                        in_=psv.rearrange("p (h c) -> p h c", c=HD),
                    )
                    yield

            def oproj_steps(it):
                """Out-projection for i-tile `it` (needs hoT cols of it)."""
                isl = slice(TI * it, TI * it + TI)
                for mo in range(NE):
                    ps = auxp.tile([P, TI], F32, tag="aux", name="op")
                    for ki in range(NM):
                        nc.tensor.matmul(
                            ps, woT[ki][:, P * mo : P * mo + P], hoT[ki][:, isl],
                            start=(ki == 0), stop=(ki == NM - 1),
                        )
                    ev = pop.tile([P, TI], F32, tag="po", name="po")
                    nc.vector.tensor_copy(out=ev, in_=ps)
                    nc.sync.dma_start(out=poT_d[P * mo : P * mo + P, isl], in_=ev)
                    yield

            def drain(gens):
                for gen in gens:
                    for _ in gen:
                        pass
                gens.clear()

            def step(gens, k=1):
                # advance the first non-exhausted generator by k chunks
                for _ in range(k):
                    while gens:
                        if next(gens[0], "done") == "done":
                            gens.pop(0)
                        else:
                            break

            # ---- prologue: projections for i-tile 0
            drain([proj_steps(0)])

            # ---- main loop: attention(it), with projections for it+1 and
            # deferred out-projections of earlier i-tiles spread into the
            # groups as independent PE filler work
            fillers = []
            for it in range(NT):
                isl = slice(TI * it, TI * it + TI)
                njt = 4 * (it + 1)
                proj_gen = [proj_steps(it + 1)] if it + 1 < NT else []
                for hp in range(NM):
                    m = hp
                    outps = [
                        outp.tile([P, TI], F32, tag="out", name=f"out{hx}")
                        for hx in range(2)
                    ]
                    for g in range(njt // 2):
                        jts = (2 * g, 2 * g + 1)
                        scs = [
                            scp.tile([P, 2 * TI], F32, tag="sc", name=f"sc{hx}")
                            for hx in range(2)
                        ]
                        # row-packed score matmuls: heads at base partitions 0/64
                        for jj, jt in enumerate(jts):
                            for hx in range(2):
                                bp = 64 * hx
                                nc.tensor.matmul(
                                    scs[hx][:, TI * jj : TI * jj + TI],
                                    kT[m][bp : bp + 64, P * jt : P * jt + P],
                                    qT[m][bp : bp + 64, isl],
                                    start=True, stop=True,
                                )
                        # causal mask (additive -1e9) on partial j-tiles
                        for jj, jt in enumerate(jts):
                            s = jt - 4 * it
                            if 0 <= s <= 3:
                                for hx in range(2):
                                    slc = scs[hx][:, TI * jj : TI * jj + TI]
                                    nc.vector.tensor_add(out=slc, in0=slc, in1=mkt[s])
                        # exp over the 2-jt span
                        zs = []
                        for hx in range(2):
                            z = zp.tile([P, 2 * TI], F32R, tag="z", name="z")
                            nc.scalar.activation(out=z, in_=scs[hx], func=EXP, scale=SCALE)
                            zs.append(z)
                        # PV accumulate (+ ones column -> denominator row HD)
                        for jj, jt in enumerate(jts):
                            for hx in range(2):
                                h = 2 * hp + hx
                                nc.tensor.matmul(
                                    outps[hx][0 : HD + 1, :],
                                    vg[jt].rearrange("p h c -> p (h c)")[:, 65 * h : 65 * h + 65],
                                    zs[hx][:, TI * jj : TI * jj + TI],
                                    start=(g == 0 and jj == 0),
                                    stop=(g == njt // 2 - 1 and jj == 1),
                                )
                        # independent projection/out-proj work to keep the PE warm
                        if proj_gen:
                            step(proj_gen)
                        else:
                            step(fillers)
                    # normalize: evict PSUM fast via ACT copy (frees the bank
                    # for the next head ~5us earlier), then the reciprocal
                    # chain runs off the PE critical path from SBUF.
                    for hx in range(2):
                        uo = np_.tile([HD + 1, TI], F32, tag="uo", name="uo")
                        nc.scalar.activation(out=uo, in_=outps[hx][0 : HD + 1, :], func=COPY)
                        rec = np_.tile([1, TI], F32, tag="rec", name="rec")
                        nc.vector.reciprocal(out=rec, in_=uo[HD : HD + 1, :])
                        recb = np_.tile([HD, TI], F32, tag="recb", name="recb")
                        nc.gpsimd.partition_broadcast(recb, rec)
                        nc.vector.tensor_mul(
                            out=hoT[m][HD * hx : HD * hx + HD, isl],
                            in0=uo[0:HD, :], in1=recb,
                        )
                # projections for it+1 must land before attention(it+1) starts
                drain(proj_gen)
                # out-projection of this i-tile becomes filler for later tiles
                fillers.append(oproj_steps(it))
                if it == NT - 1:
                    drain(fillers)

    nc.finalize()
    return nc


def _get_nc():
    if "nc" not in _cache:
        _cache["nc"] = _build()
    return _cache["nc"]


def _masks():
    # mk[s][p, f] = 0 if f >= p + 128*s else -1e9   (allowed iff j <= i)
    f = np.arange(TI)[None, :]
    p = np.arange(P)[:, None]
    out = np.zeros((4 * P, TI), dtype=np.float32)
    for s in range(4):
        out[P * s : P * s + P] = np.where(f >= p + P * s, 0.0, NEG)
    return out


def kernel(x, mask, context_vector, w_q, w_k, w_v, w_o, b_o, context_bias):
    from concourse.bass_utils import run_bass_kernel_spmd

    x = np.asarray(x, dtype=np.float32)
    context_vector = np.asarray(context_vector, dtype=np.float32)
    w_q = np.asarray(w_q, dtype=np.float32)
    w_k = np.asarray(w_k, dtype=np.float32)
    w_v = np.asarray(w_v, dtype=np.float32)
    w_o = np.asarray(w_o, dtype=np.float32)
    b_o = np.asarray(b_o, dtype=np.float32)
    context_bias = np.asarray(context_bias, dtype=np.float32)

    nc = _get_nc()
    mk = _masks()
    ones = np.ones((P, HPC), dtype=np.float32)
    bcv = context_bias[None, None, :] * context_vector  # [B, T, E]

    in_maps = []
    for c in range(NCORES):
        b, g = divmod(c, GROUPS)
        S = slice(C * g, C * g + C)
        in_maps.append({
            "xT": np.ascontiguousarray(x[b, :T].T),
            "wqT": np.ascontiguousarray(w_q[S, :].T),
            "wkT": np.ascontiguousarray(w_k[S, :].T),
            "wvT": np.ascontiguousarray(w_v[S, :].T),
            "woT": np.ascontiguousarray(w_o[:, S].T),
            "cvT": np.ascontiguousarray(bcv[b, :T, S].T),
            "mk": mk,
            "ones": ones,
        })

    res = run_bass_kernel_spmd(
        nc, in_maps, list(range(NCORES)),
        trace=bool(int(os.environ.get("ATTN_TRACE", "0"))),
    )
    _cache["last_results"] = res

    out = np.zeros((B, T, E), dtype=np.float32)
    for c in range(NCORES):
        b = c // GROUPS
        out[b] += res.results[c]["poT"].T
    out += b_o[None, None, :]
    return out


# revision 16
# speedup vs baseline: 1.1890x; 1.0045x over previous
"""Directed multi-head attention on 8 trn2 NeuronCores.

Sharding: 2-way batch x 4-way head tensor parallel. Core c handles batch
c//4 and heads [4*(c%4), 4*(c%4)+4) (256 of 1024 channels). Each core
computes its heads' q/k/v projections, causal attention, and a partial
out-projection; the host sums the 4 partials per batch (the all-reduce)
and adds b_o.

Layouts are chosen so every matmul contracts along the partition dim:
the host passes x^T, wq^T, wk^T, wv^T and w_o[:,S]^T per core. Scores are
computed transposed (keys on partitions, queries on free) so softmax's
denominator comes from a ones-column appended to V in the PV matmul, and
exp runs on [128, 1024] PSUM spans. All matmul operands are float32r
(tf32-like) which streams at full PE rate for free dims >= 256.

The q/k/v projections are software-pipelined INTO the attention i-tile
loop (projections for i-tile n+1 are emitted between attention groups of
i-tile n) so the PE always has independent work while the Scalar engine
runs exp -- otherwise the PE HAM clock-gate re-throttles to 1.2 GHz
during ACT-bound stretches and doubles every matmul.
"""

import os

import numpy as np

E = 1024
H = 16
HD = 64
B = 2
NCORES = 8
GROUPS = NCORES // B  # head-parallel degree per batch
HPC = H // GROUPS  # heads per core
C = HPC * HD  # channels per core
P = 128
TI = 512  # i-tile (query) width, max fp32 matmul free dim

T = int(os.environ.get("ATTN_T", "2048"))
NEG = -1e9
SCALE = 1.0 / 8.0  # 1/sqrt(HD)

_cache = {}


def _build():
    import concourse.bacc as bacc
    import concourse.mybir as mybir
    from concourse.tile import TileContext

    F32 = mybir.dt.float32
    F32R = mybir.dt.float32r
    EXP = mybir.ActivationFunctionType.Exp
    COPY = mybir.ActivationFunctionType.Copy

    NT = T // TI  # i-tiles
    NJ = T // P  # j-tiles
    NE = E // P  # embed k-tiles
    NM = C // P  # channel partition tiles (2)

    nc = bacc.Bacc(debug=False)
    xT_d = nc.dram_tensor("xT", [E, T], F32R, kind="ExternalInput")
    wqT_d = nc.dram_tensor("wqT", [E, C], F32R, kind="ExternalInput")
    wkT_d = nc.dram_tensor("wkT", [E, C], F32R, kind="ExternalInput")
    wvT_d = nc.dram_tensor("wvT", [E, C], F32R, kind="ExternalInput")
    woT_d = nc.dram_tensor("woT", [C, E], F32R, kind="ExternalInput")
    cvT_d = nc.dram_tensor("cvT", [C, T], F32, kind="ExternalInput")
    mk_d = nc.dram_tensor("mk", [4 * P, TI], F32, kind="ExternalInput")
    ones_d = nc.dram_tensor("ones", [P, HPC], F32R, kind="ExternalInput")
    poT_d = nc.dram_tensor("poT", [E, T], F32, kind="ExternalOutput")

    with TileContext(nc) as tc:
        with (
            tc.tile_pool(name="persist", bufs=1) as pp,
            tc.tile_pool(name="xs", bufs=2) as xsp,
            tc.tile_pool(name="cv_in", bufs=3) as cvp,
            tc.tile_pool(name="zp", bufs=4) as zp,
            tc.tile_pool(name="np_", bufs=4) as np_,
            tc.tile_pool(name="po", bufs=4) as pop,
            tc.tile_pool(name="sc_ps", bufs=2, space="PSUM") as scp,
            tc.tile_pool(name="out_ps", bufs=2, space="PSUM") as outp,
            tc.tile_pool(name="aux_ps", bufs=2, space="PSUM") as auxp,
        ):
            # ---- persistent tiles
            qT = [pp.tile([P, T], F32R, tag=f"qT{m}", name=f"qT{m}") for m in range(NM)]
            kT = [pp.tile([P, T], F32R, tag=f"kT{m}", name=f"kT{m}") for m in range(NM)]
            vg = [pp.tile([P, HPC, HD + 1], F32R, tag=f"vg{j}", name=f"vg{j}") for j in range(NJ)]
            hoT = [pp.tile([P, T], F32R, tag=f"hoT{m}", name=f"hoT{m}") for m in range(NM)]
            mkt = [pp.tile([P, TI], F32, tag=f"mk{s}", name=f"mk{s}") for s in range(4)]
            woT = [pp.tile([P, E], F32R, tag=f"woT{m}", name=f"woT{m}") for m in range(NM)]
            wq = [pp.tile([P, C], F32R, tag=f"wq{e}", name=f"wq{e}") for e in range(NE)]
            wk = [pp.tile([P, C], F32R, tag=f"wk{e}", name=f"wk{e}") for e in range(NE)]
            wv = [pp.tile([P, C], F32R, tag=f"wv{e}", name=f"wv{e}") for e in range(NE)]

            # weights on the scalar HWDGE queue, streaming data on sync, tiny
            # ones-column writes on gpsimd SWDGE: the prologue was previously
            # ~40us of serialized DMA on one queue.
            for e in range(NE):
                nc.scalar.dma_start(out=wq[e], in_=wqT_d[P * e : P * e + P, :])
                nc.scalar.dma_start(out=wk[e], in_=wkT_d[P * e : P * e + P, :])
                nc.scalar.dma_start(out=wv[e], in_=wvT_d[P * e : P * e + P, :])
            for j in range(NJ):
                nc.gpsimd.dma_start(out=vg[j][:, :, HD : HD + 1], in_=ones_d[:, :].unsqueeze(2))
            for s in range(4):
                nc.scalar.dma_start(out=mkt[s], in_=mk_d[P * s : P * s + P, :])
            for m in range(NM):
                nc.scalar.dma_start(out=woT[m], in_=woT_d[P * m : P * m + P, :])
            # warm the ACT exp spline table before the first real exp
            warm = pp.tile([1, 16], F32, tag="warm", name="warm")
            nc.vector.memset(warm, 0.0)
            warm2 = pp.tile([1, 16], F32, tag="warm2", name="warm2")
            nc.scalar.activation(out=warm2, in_=warm, func=EXP)

            # ---- projection emitters (generators: one yield = one PE-chunk)
            def proj_steps(n):
                """Emit q/k/v projections for token window n (cols TI*n..) in
                small chunks; yields between chunks so the caller can
                interleave them with attention groups."""
                ns = slice(TI * n, TI * n + TI)
                xs = [xsp.tile([P, TI], F32R, tag=f"xs{e}", name=f"xs{e}") for e in range(NE)]
                for e in range(NE):
                    nc.sync.dma_start(out=xs[e], in_=xT_d[P * e : P * e + P, ns])
                yield
                for m in range(NM):
                    cvt = cvp.tile([P, TI], F32, tag="cv", name="cv")
                    nc.sync.dma_start(out=cvt, in_=cvT_d[P * m : P * m + P, ns])
                    psq = auxp.tile([P, TI], F32, tag="aux", name="psq")
                    for e in range(NE):
                        nc.tensor.matmul(
                            psq, wq[e][:, P * m : P * m + P], xs[e],
                            start=(e == 0), stop=(e == NE - 1),
                        )
                    nc.vector.tensor_add(out=qT[m][:, ns], in0=psq, in1=cvt)
                    yield
                    psk = auxp.tile([P, TI], F32, tag="aux", name="psk")
                    for e in range(NE):
                        nc.tensor.matmul(
                            psk, wk[e][:, P * m : P * m + P], xs[e],
                            start=(e == 0), stop=(e == NE - 1),
                        )
                    nc.vector.tensor_copy(out=kT[m][:, ns], in_=psk)
                    yield
                for jj in range(TI // P):
                    j = (TI * n) // P + jj
                    psv = auxp.tile([P, C], F32, tag="aux", name="psv")
                    for e in range(NE):
                        nc.tensor.matmul(
                            psv, xs[e][:, P * jj : P * jj + P], wv[e],
                            start=(e == 0), stop=(e == NE - 1),
                        )
                    nc.vector.tensor_copy(
                        out=vg[j][:, :, 0:HD],
                        in_=psv.rearrange("p (h c) -> p h c", c=HD),
                    )
                    yield

            def oproj_steps(it):
                """Out-projection for i-tile `it` (needs hoT cols of it)."""
                isl = slice(TI * it, TI * it + TI)
                for mo in range(NE):
                    ps = auxp.tile([P, TI], F32, tag="aux", name="op")
                    for ki in range(NM):
                        nc.tensor.matmul(
                            ps, woT[ki][:, P * mo : P * mo + P], hoT[ki][:, isl],
                            start=(ki == 0), stop=(ki == NM - 1),
                        )
                    ev = pop.tile([P, TI], F32, tag="po", name="po")
                    nc.vector.tensor_copy(out=ev, in_=ps)
                    nc.sync.dma_start(out=poT_d[P * mo : P * mo + P, isl], in_=ev)
                    yield

            def drain(gens):
                for gen in gens:
                    for _ in gen:
                        pass
                gens.clear()

            def step(gens, k=1):
                # advance the first non-exhausted generator by k chunks
                for _ in range(k):
                    while gens:
                        if next(gens[0], "done") == "done":
                            gens.pop(0)
                        else:
                            break

            # ---- prologue: projections for i-tile 0
            drain([proj_steps(0)])

            # ---- main loop: attention(it), with projections for it+1 and
            # deferred out-projections of earlier i-tiles spread into the
            # groups as independent PE filler work
            fillers = []
            for it in range(NT):
                isl = slice(TI * it, TI * it + TI)
                njt = 4 * (it + 1)
                proj_gen = [proj_steps(it + 1)] if it + 1 < NT else []
                for hp in range(NM):
                    m = hp
                    outps = [
                        outp.tile([P, TI], F32, tag="out", name=f"out{hx}")
                        for hx in range(2)
                    ]
                    for g in range(njt // 2):
                        jts = (2 * g, 2 * g + 1)
                        scs = [
                            scp.tile([P, 2 * TI], F32, tag="sc", name=f"sc{hx}")
                            for hx in range(2)
                        ]
                        # row-packed score matmuls: heads at base partitions 0/64
                        for jj, jt in enumerate(jts):
                            for hx in range(2):
                                bp = 64 * hx
                                nc.tensor.matmul(
                                    scs[hx][:, TI * jj : TI * jj + TI],
                                    kT[m][bp : bp + 64, P * jt : P * jt + P],
                                    qT[m][bp : bp + 64, isl],
                                    start=True, stop=True,
                                )
                        # causal mask (additive -1e9) on partial j-tiles
                        for jj, jt in enumerate(jts):
                            s = jt - 4 * it
                            if 0 <= s <= 3:
                                for hx in range(2):
                                    slc = scs[hx][:, TI * jj : TI * jj + TI]
                                    nc.vector.tensor_add(out=slc, in0=slc, in1=mkt[s])
                        # exp over the 2-jt span
                        zs = []
                        for hx in range(2):
                            z = zp.tile([P, 2 * TI], F32R, tag="z", name="z")
                            nc.scalar.activation(out=z, in_=scs[hx], func=EXP, scale=SCALE)
                            zs.append(z)
                        # PV accumulate (+ ones column -> denominator row HD)
                        for jj, jt in enumerate(jts):
                            for hx in range(2):
                                h = 2 * hp + hx
                                nc.tensor.matmul(
                                    outps[hx][0 : HD + 1, :],
                                    vg[jt].rearrange("p h c -> p (h c)")[:, 65 * h : 65 * h + 65],
                                    zs[hx][:, TI * jj : TI * jj + TI],
                                    start=(g == 0 and jj == 0),
                                    stop=(g == njt // 2 - 1 and jj == 1),
                                )
                        # independent projection/out-proj work to keep the PE warm
                        if proj_gen:
                            step(proj_gen)
                        else:
                            step(fillers)
                    # normalize: evict PSUM fast via ACT copy (frees the bank
                    # for the next head ~5us earlier), then the reciprocal
                    # chain runs off the PE critical path from SBUF.
                    for hx in range(2):
                        uo = np_.tile([HD + 1, TI], F32, tag="uo", name="uo")
                        nc.scalar.activation(out=uo, in_=outps[hx][0 : HD + 1, :], func=COPY)
                        rec = np_.tile([1, TI], F32, tag="rec", name="rec")
                        nc.vector.reciprocal(out=rec, in_=uo[HD : HD + 1, :])
                        recb = np_.tile([HD, TI], F32, tag="recb", name="recb")
                        nc.gpsimd.partition_broadcast(recb, rec)
                        nc.vector.tensor_mul(
                            out=hoT[m][HD * hx : HD * hx + HD, isl],
                            in0=uo[0:HD, :], in1=recb,
                        )
                        # keep the PE fed while the normalize chain drains
                        if proj_gen:
                            step(proj_gen)
                        else:
                            step(fillers)
                # projections for it+1 must land before attention(it+1) starts
                drain(proj_gen)
                # out-projection of this i-tile becomes filler for later tiles
                fillers.append(oproj_steps(it))
                if it == NT - 1:
                    drain(fillers)

    nc.finalize()
    return nc


def _get_nc():
    if "nc" not in _cache:
        _cache["nc"] = _build()
    return _cache["nc"]


def _masks():
    # mk[s][p, f] = 0 if f >= p + 128*s else -1e9   (allowed iff j <= i)
    f = np.arange(TI)[None, :]
    p = np.arange(P)[:, None]
    out = np.zeros((4 * P, TI), dtype=np.float32)
    for s in range(4):
        out[P * s : P * s + P] = np.where(f >= p + P * s, 0.0, NEG)
    return out


def kernel(x, mask, context_vector, w_q, w_k, w_v, w_o, b_o, context_bias):
    from concourse.bass_utils import run_bass_kernel_spmd

    x = np.asarray(x, dtype=np.float32)
    context_vector = np.asarray(context_vector, dtype=np.float32)
    w_q = np.asarray(w_q, dtype=np.float32)
    w_k = np.asarray(w_k, dtype=np.float32)
    w_v = np.asarray(w_v, dtype=np.float32)
    w_o = np.asarray(w_o, dtype=np.float32)
    b_o = np.asarray(b_o, dtype=np.float32)
    context_bias = np.asarray(context_bias, dtype=np.float32)

    nc = _get_nc()
    mk = _masks()
    ones = np.ones((P, HPC), dtype=np.float32)
    bcv = context_bias[None, None, :] * context_vector  # [B, T, E]

    in_maps = []
    for c in range(NCORES):
        b, g = divmod(c, GROUPS)
        S = slice(C * g, C * g + C)
        in_maps.append({
            "xT": np.ascontiguousarray(x[b, :T].T),
            "wqT": np.ascontiguousarray(w_q[S, :].T),
            "wkT": np.ascontiguousarray(w_k[S, :].T),
            "wvT": np.ascontiguousarray(w_v[S, :].T),
            "woT": np.ascontiguousarray(w_o[:, S].T),
            "cvT": np.ascontiguousarray(bcv[b, :T, S].T),
            "mk": mk,
            "ones": ones,
        })

    res = run_bass_kernel_spmd(
        nc, in_maps, list(range(NCORES)),
        trace=bool(int(os.environ.get("ATTN_TRACE", "0"))),
    )
    _cache["last_results"] = res

    out = np.zeros((B, T, E), dtype=np.float32)
    for c in range(NCORES):
        b = c // GROUPS
        out[b] += res.results[c]["poT"].T
    out += b_o[None, None, :]
    return out
